# revision 28
# baseline (speedup 1.0000x reference)
"""Trainium2 Bass kernel for nn_BiLinearInteractionLayer (fp16 fast path).

Math: x:(B=4096, F=32, D=64) f32, W:(P=496, D=64, D=64) f32 (torch Linear
layout).  For each pair p=(i,j), i<j:
    out[b, p, e] = (sum_d x[b,i,d] * W[p,e,d]) * x[b,j,e]

The harness gate is rel_err < 2e-2 (max-abs / max-scale).  The original
kernel computed to 2.7e-7 with an exact hi/lo fp16 expansion and stored
fp32 output -- but it is HBM-bound (65 of 77 MB/core is the output
store).  This version computes in fp16 (~8e-4 rel err, ~25x inside the
gate) and halves the dominant traffic:

  per core: x fp16 2MB + xT fp16 2MB + W^T fp16 3.9MB + out fp16 32.5MB
  = 40.4MB vs 77MB before, at the ~358 GB/s HBM-per-core limit.

Design (data-parallel over batch, 8 cores x 512 rows):

* Host precomputes fp16 x in natural layout, fp16 x^T per batch tile in
  a per-field-pair layout (field 2g on rows 0:63, field 2g+1 on rows
  64:127), and fp16 W^T -- no on-chip transposes.
* Matmuls are k=128: the stationary is one PAIR of fields' x^T, and the
  streamed weight tile has the other field's 64 rows zeroed.  k=128
  keeps the PE HAM monitor un-throttled at 2.4 GHz (k=64 under-reports
  and pins 1.2 GHz -- measured +35us), and LDWEIGHTS overlaps matmuls.
  Row-group packing (tile_position) was rejected: two concurrent
  row-group matmuls draining into one PSUM bank are a fatal HW error
  (verified by bisection) and the output layout can't keep them apart.
* Weights live in 4 SBUF tiles (even/odd fields x groups 0-3 / 4-15,
  data on partitions 0:63 / 64:127 resp.), each zero-half initialized
  by ONE big DVE memset (~3us) and filled by ONE DMA.  An earlier
  variant used 31 per-group memsets + 31 DMAs: the DVE DRAIN per memset
  plus the sync ring's ~620ns per-DMA issue cost ~35us of startup
  serialization.  Splitting even/odd at group 4 gives window 0 its
  weights after ~2 DMAs while the rest stream in behind.
* TRN2 matmul can only write fp32 PSUM.  Evacuation + elementwise
  multiply, per 2048-col window (4 PSUM banks):
    path B (most windows): ACT copies PSUM->SBUF fp16 ((172+FD)/1.2GHz),
      then DVE tensor_mul fp16*fp16->fp16 at 2x_1P.
    path A (PATH_A windows): DVE multiplies straight from fp32 PSUM at
      1x (mixed-dtype tensor_tensor with an fp16 SBUF operand works).
  The split keeps ACT and DVE balanced under the DMA floor.
* All loads AND stores go on the Sync HWDGE ring; stores must not be
  issued from ACT (a store waiting on DVE in the ACT FIFO queue blocks
  the next window's PSUM copy -- cost ~25us in an earlier variant).
  bt+1's loads are issued before bt's stores so the FIFO ring never
  parks a ready load behind a store that waits on compute.
"""
import numpy as np
from bisect import bisect_right

import concourse.bacc as bacc
import concourse.tile as tile
import concourse.mybir as mybir
from concourse.bass_utils import run_bass_kernel_spmd

B = 4096
F = 32
D = 64
P = F * (F - 1) // 2  # 496
N_CORES = 8
BL = B // N_CORES     # 512 rows per core
BT = 128              # batch tile (SBUF partitions)
NBT = BL // BT        # 4 batch tiles per core
NCOL = P * D          # 31744 output columns per row
NGRP = F // 2         # 16 field-pair groups
GSPLIT = 4            # weight tiles split at this group
WIN = 2048            # evacuation window = 4 fp32 PSUM banks
NWIN = (NCOL + WIN - 1) // WIN  # 16 (last window 1024)
PATH_A = (1, 5, 10, 15)  # windows multiplied straight from PSUM (DVE 1x)
GPS_W = (3, 8, 12)       # path-B windows whose muls run on GPSIMD

f32 = mybir.dt.float32
f16 = mybir.dt.float16

_nc_cache = None


def _off(i):
    """Number of pairs with left field < i."""
    return 31 * i - i * (i - 1) // 2


_FLD_START = [_off(i) * D for i in range(F)]            # canonical col starts
_FLD_W = [(31 - i) * D for i in range(F)]               # field block widths
# weight DRAM layout: even-field blocks in group order, then odd-field
# blocks in group order; _FLD_BASE = col of each field inside its e/o block
_FLD_BASE = [0] * F
for _f in range(2, F):
    _FLD_BASE[_f] = _FLD_BASE[_f - 2] + _FLD_W[_f - 2]
_EW = sum(_FLD_W[0::2])                                 # 16384 even cols
_OW = sum(_FLD_W[1::2])                                 # 15360 odd cols
_EA = _FLD_BASE[2 * GSPLIT]                             # even split col
_OA = _FLD_BASE[2 * GSPLIT + 1]                         # odd split col
_MM_BOUNDS = sorted(set(range(0, NCOL, 512)) | set(_FLD_START))
_FLD_BOUNDS = _FLD_START[1:]


def _segments(w0, w1, bounds):
    pts = [w0] + [b for b in bounds if w0 < b < w1] + [w1]
    return list(zip(pts[:-1], pts[1:]))


def _build():
    nc = bacc.Bacc("TRN2", target_bir_lowering=False, debug=False,
                   num_devices=N_CORES)
    x_in = nc.dram_tensor("x16", [BL, F * D], f16, kind="ExternalInput").ap()
    xt_in = nc.dram_tensor("xt", [128, NBT * NGRP * BT], f16,
                           kind="ExternalInput").ap()
    # cols [0:_EW) = even-field blocks, [_EW:) = odd-field blocks
    wt_in = nc.dram_tensor("wt", [D, _EW + _OW], f16,
                           kind="ExternalInput").ap()
    out = nc.dram_tensor("out", [BL, NCOL], f16, kind="ExternalOutput").ap()

    with tile.TileContext(nc) as tc:
        with (
            tc.tile_pool(name="consts", bufs=1) as consts,
            tc.tile_pool(name="xp", bufs=2) as xp,
            tc.tile_pool(name="xtp", bufs=2) as xtp,
            tc.tile_pool(name="mmp", bufs=3) as mmp,
            tc.tile_pool(name="otp", bufs=3) as otp,
            tc.tile_pool(name="psm", bufs=2, space="PSUM") as psm,
        ):
            # 4 weight tiles: (even|odd fields) x (groups 0:3 | 4:15).
            # Even-field data on partitions 0:63 (matching x^T rows), odd
            # on 64:127; the complementary half-rows are zero.  Shipping
            # the zeros from DRAM instead costs +4MB on the bottleneck
            # DMA (+13us measured) -- memsets are cheaper.
            tEa = consts.tile([128, _EA], f16, tag="wEa")
            tEb = consts.tile([128, _EW - _EA], f16, tag="wEb")
            tOa = consts.tile([128, _OA], f16, tag="wOa")
            tOb = consts.tile([128, _OW - _OA], f16, tag="wOb")
            # fp16 memset runs at 1x; bitcast to f32 halves the element
            # count.  Even-tile zeros on DVE, odd-tile zeros on the (idle)
            # GPSIMD engine so both halves clear concurrently.
            nc.vector.memset(tEa[64:128, :].bitcast(f32), 0.0)
            nc.gpsimd.memset(tOa[0:64, :].bitcast(f32), 0.0)
            nc.vector.memset(tEb[64:128, :].bitcast(f32), 0.0)
            nc.gpsimd.memset(tOb[0:64, :].bitcast(f32), 0.0)

            def wslice(f, lc, n):
                """Weight-tile slice for cols [lc, lc+n) of field f's block."""
                b = _FLD_BASE[f]
                if f % 2 == 0:
                    t, b0 = (tEa, 0) if b < _EA else (tEb, _EA)
                    return t[:, b - b0 + lc:b - b0 + lc + n]
                t, b0 = (tOa, 0) if b < _OA else (tOb, _OA)
                return t[:, b - b0 + lc:b - b0 + lc + n]

            def load_bt(bt):
                x16 = xp.tile([BT, F * D], f16, tag="x")
                nc.sync.dma_start(out=x16,
                                  in_=x_in[bt * BT:(bt + 1) * BT, :])
                c0 = bt * NGRP * BT
                xT = xtp.tile([128, NGRP * BT], f16, tag="xT")
                nc.sync.dma_start(out=xT, in_=xt_in[:, c0:c0 + NGRP * BT])
                return x16, xT

            tiles = load_bt(0)
            nc.sync.dma_start(out=tEa[0:64, :], in_=wt_in[:, 0:_EA])
            nc.sync.dma_start(out=tOa[64:128, :],
                              in_=wt_in[:, _EW:_EW + _OA])
            nc.sync.dma_start(out=tEb[0:64, :], in_=wt_in[:, _EA:_EW])
            nc.sync.dma_start(out=tOb[64:128, :],
                              in_=wt_in[:, _EW + _OA:_EW + _OW])

            for bt in range(NBT):
                r0, r1 = bt * BT, (bt + 1) * BT
                x16, xT = tiles
                if bt + 1 < NBT:
                    tiles = load_bt(bt + 1)

                # stores span 4 windows (2MB) for SWDGE efficiency; the
                # last batch tile stores every 2 windows to shorten the
                # final drain after compute ends
                span = 2 if bt == NBT - 1 else 4
                ot = None
                o0 = 0
                for w in range(NWIN):
                    w0 = w * WIN
                    w1 = min(w0 + WIN, NCOL)
                    wl = w1 - w0
                    if w % span == 0:
                        o0 = w0
                        ot = otp.tile([BT, 4 * WIN], f16, tag="ot")

                    pm = psm.tile([BT, WIN], f32, tag="mm")
                    for (s0, s1) in _segments(w0, w1, _MM_BOUNDS):
                        f = bisect_right(_FLD_START, s0) - 1
                        g = f // 2
                        nc.tensor.matmul(
                            pm[:, s0 - w0:s1 - w0],
                            xT[:, g * BT:(g + 1) * BT],
                            wslice(f, s0 - _FLD_START[f], s1 - s0),
                            start=True, stop=True)

                    if w in PATH_A:
                        src = pm
                        eng = nc.vector
                    else:
                        src = mmp.tile([BT, WIN], f16, tag="m16")
                        nc.scalar.copy(src[:, :wl], pm[:, :wl])
                        eng = nc.gpsimd if w in GPS_W else nc.vector
                    for (s0, s1) in _segments(w0, w1, _FLD_BOUNDS):
                        i = bisect_right(_FLD_START, s0) - 1
                        xc = (i + 1) * D + (s0 - _FLD_START[i])
                        eng.tensor_mul(
                            ot[:, s0 - o0:s1 - o0],
                            src[:, s0 - w0:s1 - w0],
                            x16[:, xc:xc + (s1 - s0)])

                    if w % span == span - 1 or w == NWIN - 1:
                        ol = w1 - o0
                        nc.gpsimd.dma_start(out=out[r0:r1, o0:o0 + ol],
                                            in_=ot[:, :ol])
    nc.compile()
    return nc


def _get_nc():
    global _nc_cache
    if _nc_cache is None:
        _nc_cache = _build()
    return _nc_cache


def _prep_inputs(x, W):
    x16 = np.asarray(x, dtype=np.float16)            # (B, F, D)
    xs = np.ascontiguousarray(x16.reshape(N_CORES, BL, F * D))
    # xt[c, h*64+d, bt*2048 + g*128 + b] = x[c, bt*128+b, 2g+h, d]
    xr = x16.reshape(N_CORES, NBT, BT, NGRP, 2, D)
    xt = np.ascontiguousarray(xr.transpose(0, 4, 5, 1, 3, 2)).reshape(
        N_CORES, 128, NBT * NGRP * BT)
    # canonical W^T: wtc[d, p*64+e] = W[p, e, d]
    wtc = np.ascontiguousarray(
        np.asarray(W, dtype=np.float32).transpose(2, 0, 1).reshape(D, NCOL)
    ).astype(np.float16)
    # even-field blocks then odd-field blocks
    wt2 = np.empty((D, _EW + _OW), dtype=np.float16)
    for f in range(F - 1):
        b0 = _FLD_BASE[f] + (0 if f % 2 == 0 else _EW)
        wt2[:, b0:b0 + _FLD_W[f]] = \
            wtc[:, _FLD_START[f]:_FLD_START[f] + _FLD_W[f]]
    return xs, xt, wt2


def _run(x, W, trace=False, trace_kwargs=None):
    xs, xt, wt2 = _prep_inputs(x, W)
    in_maps = [{"x16": xs[c], "xt": xt[c], "wt": wt2} for c in range(N_CORES)]
    res = run_bass_kernel_spmd(_get_nc(), in_maps, list(range(N_CORES)),
                               trace=trace, **(trace_kwargs or {}))
    outs = [res.results[c]["out"].astype(np.float32).reshape(BL, P, D)
            for c in range(N_CORES)]
    return np.concatenate(outs, axis=0), res


def kernel(x, W):
    out, _ = _run(x, W)
    return out


# revision 34
# speedup vs baseline: 1.1261x; 1.1261x over previous
"""Trainium2 Bass kernel for nn_BiLinearInteractionLayer (fp16 fast path).

Math: x:(B=4096, F=32, D=64) f32, W:(P=496, D=64, D=64) f32 (torch Linear
layout).  For each pair p=(i,j), i<j:
    out[b, p, e] = (sum_d x[b,i,d] * W[p,e,d]) * x[b,j,e]

The harness gate is rel_err < 2e-2 (max-abs / max-scale).  The original
kernel computed to 2.7e-7 with an exact hi/lo fp16 expansion and stored
fp32 output -- but it is HBM-bound (65 of 77 MB/core is the output
store).  This version computes in fp16 (~8e-4 rel err, ~25x inside the
gate) and halves the dominant traffic:

  per core: x fp16 2MB + xT fp16 2MB + W^T fp16 3.9MB + out fp16 32.5MB
  = 40.4MB vs 77MB before, at the ~358 GB/s HBM-per-core limit.

Design (data-parallel over batch, 8 cores x 512 rows):

* Host precomputes fp16 x in natural layout, fp16 x^T per batch tile in
  a per-field-pair layout (field 2g on rows 0:63, field 2g+1 on rows
  64:127), and fp16 W^T -- no on-chip transposes.
* Matmuls are k=128: the stationary is one PAIR of fields' x^T, and the
  streamed weight tile has the other field's 64 rows zeroed.  k=128
  keeps the PE HAM monitor un-throttled at 2.4 GHz (k=64 under-reports
  and pins 1.2 GHz -- measured +35us), and LDWEIGHTS overlaps matmuls.
  Row-group packing (tile_position) was rejected: two concurrent
  row-group matmuls draining into one PSUM bank are a fatal HW error
  (verified by bisection) and the output layout can't keep them apart.
* Weights live in 4 SBUF tiles (even/odd fields x groups 0-3 / 4-15,
  data on partitions 0:63 / 64:127 resp.), each zero-half initialized
  by ONE big DVE memset (~3us) and filled by ONE DMA.  An earlier
  variant used 31 per-group memsets + 31 DMAs: the DVE DRAIN per memset
  plus the sync ring's ~620ns per-DMA issue cost ~35us of startup
  serialization.  Splitting even/odd at group 4 gives window 0 its
  weights after ~2 DMAs while the rest stream in behind.
* TRN2 matmul can only write fp32 PSUM.  Evacuation + elementwise
  multiply, per 2048-col window (4 PSUM banks):
    path B (most windows): ACT copies PSUM->SBUF fp16 ((172+FD)/1.2GHz),
      then DVE tensor_mul fp16*fp16->fp16 at 2x_1P.
    path A (PATH_A windows): DVE multiplies straight from fp32 PSUM at
      1x (mixed-dtype tensor_tensor with an fp16 SBUF operand works).
  The split keeps ACT and DVE balanced under the DMA floor.
* All loads AND stores go on the Sync HWDGE ring; stores must not be
  issued from ACT (a store waiting on DVE in the ACT FIFO queue blocks
  the next window's PSUM copy -- cost ~25us in an earlier variant).
  bt+1's loads are issued before bt's stores so the FIFO ring never
  parks a ready load behind a store that waits on compute.
"""
import numpy as np
from bisect import bisect_right

import concourse.bacc as bacc
import concourse.tile as tile
import concourse.mybir as mybir
from concourse.bass_utils import run_bass_kernel_spmd

B = 4096
F = 32
D = 64
P = F * (F - 1) // 2  # 496
N_CORES = 8
BL = B // N_CORES     # 512 rows per core
BT = 128              # batch tile (SBUF partitions)
NBT = BL // BT        # 4 batch tiles per core
NCOL = P * D          # 31744 output columns per row
NGRP = F // 2         # 16 field-pair groups
GSPLIT = 4            # weight tiles split at this group
WIN = 2048            # evacuation window = 4 fp32 PSUM banks
NWIN = (NCOL + WIN - 1) // WIN  # 16 (last window 1024)
PATH_A = (4, 9, 14)   # windows multiplied straight from PSUM (DVE 1x)
GPS_W = ()            # gpsimd mul offload: measured WORSE (183 vs 159us)

f32 = mybir.dt.float32
f16 = mybir.dt.float16

_nc_cache = None


def _off(i):
    """Number of pairs with left field < i."""
    return 31 * i - i * (i - 1) // 2


_FLD_START = [_off(i) * D for i in range(F)]            # canonical col starts
_FLD_W = [(31 - i) * D for i in range(F)]               # field block widths
# weight DRAM layout: even-field blocks in group order, then odd-field
# blocks in group order; _FLD_BASE = col of each field inside its e/o block
_FLD_BASE = [0] * F
for _f in range(2, F):
    _FLD_BASE[_f] = _FLD_BASE[_f - 2] + _FLD_W[_f - 2]
_EW = sum(_FLD_W[0::2])                                 # 16384 even cols
_OW = sum(_FLD_W[1::2])                                 # 15360 odd cols
_EA = _FLD_BASE[2 * GSPLIT]                             # even split col
_OA = _FLD_BASE[2 * GSPLIT + 1]                         # odd split col
_MM_BOUNDS = sorted(set(range(0, NCOL, 512)) | set(_FLD_START))
_FLD_BOUNDS = _FLD_START[1:]


def _segments(w0, w1, bounds):
    pts = [w0] + [b for b in bounds if w0 < b < w1] + [w1]
    return list(zip(pts[:-1], pts[1:]))


def _build():
    nc = bacc.Bacc("TRN2", target_bir_lowering=False, debug=False,
                   num_devices=N_CORES)
    x_in = nc.dram_tensor("x16", [BL, F * D], f16, kind="ExternalInput").ap()
    xt_in = nc.dram_tensor("xt", [128, NBT * NGRP * BT], f16,
                           kind="ExternalInput").ap()
    # groups 0:3 ship zero-padded (full 128 rows) so window 0's weights
    # need no memset->DMA chain; cols [0:_EA) even, [_EA:) odd
    wta_in = nc.dram_tensor("wta", [128, _EA + _OA], f16,
                            kind="ExternalInput").ap()
    # groups 4:15 unpadded: cols [0:_EW-_EA) even, then odd
    wtb_in = nc.dram_tensor("wtb", [D, (_EW - _EA) + (_OW - _OA)], f16,
                            kind="ExternalInput").ap()
    out = nc.dram_tensor("out", [BL, NCOL], f16, kind="ExternalOutput").ap()

    with tile.TileContext(nc) as tc:
        with (
            tc.tile_pool(name="consts", bufs=1) as consts,
            tc.tile_pool(name="xp", bufs=2) as xp,
            tc.tile_pool(name="xtp", bufs=2) as xtp,
            tc.tile_pool(name="mmp", bufs=3) as mmp,
            tc.tile_pool(name="otp", bufs=3) as otp,
            tc.tile_pool(name="psm", bufs=2, space="PSUM") as psm,
        ):
            # 4 weight tiles: (even|odd fields) x (groups 0:3 | 4:15).
            # Even-field data on partitions 0:63 (matching x^T rows), odd
            # on 64:127; the complementary half-rows are zero.  Shipping
            # the zeros from DRAM instead costs +4MB on the bottleneck
            # DMA (+13us measured) -- memsets are cheaper.
            tEa = consts.tile([128, _EA], f16, tag="wEa")
            tEb = consts.tile([128, _EW - _EA], f16, tag="wEb")
            tOa = consts.tile([128, _OA], f16, tag="wOa")
            tOb = consts.tile([128, _OW - _OA], f16, tag="wOb")
            # fp16 memset runs at 1x; bitcast to f32 halves the element
            # count.  Even-tile zeros on DVE, odd-tile zeros on the (idle)
            # GPSIMD engine so both halves clear concurrently.  (a-tiles
            # arrive pre-padded from DRAM: the early DMA timeline is
            # starved, so their zeros ride free.)
            nc.vector.memset(tEb[64:128, :].bitcast(f32), 0.0)
            nc.gpsimd.memset(tOb[0:64, :].bitcast(f32), 0.0)

            def wslice(f, lc, n):
                """Weight-tile slice for cols [lc, lc+n) of field f's block."""
                b = _FLD_BASE[f]
                if f % 2 == 0:
                    t, b0 = (tEa, 0) if b < _EA else (tEb, _EA)
                    return t[:, b - b0 + lc:b - b0 + lc + n]
                t, b0 = (tOa, 0) if b < _OA else (tOb, _OA)
                return t[:, b - b0 + lc:b - b0 + lc + n]

            def load_bt(bt):
                x16 = xp.tile([BT, F * D], f16, tag="x")
                nc.sync.dma_start(out=x16,
                                  in_=x_in[bt * BT:(bt + 1) * BT, :])
                c0 = bt * NGRP * BT
                xT = xtp.tile([128, NGRP * BT], f16, tag="xT")
                nc.sync.dma_start(out=xT, in_=xt_in[:, c0:c0 + NGRP * BT])
                return x16, xT

            tiles = load_bt(0)
            nc.sync.dma_start(out=tEa, in_=wta_in[:, 0:_EA])
            nc.sync.dma_start(out=tOa, in_=wta_in[:, _EA:_EA + _OA])
            nc.sync.dma_start(out=tEb[0:64, :], in_=wtb_in[:, 0:_EW - _EA])
            nc.sync.dma_start(
                out=tOb[64:128, :],
                in_=wtb_in[:, _EW - _EA:(_EW - _EA) + (_OW - _OA)])

            for bt in range(NBT):
                r0, r1 = bt * BT, (bt + 1) * BT
                x16, xT = tiles
                if bt + 1 < NBT:
                    tiles = load_bt(bt + 1)

                # stores span 4 windows (2MB) for SWDGE efficiency; the
                # last batch tile stores every 2 windows to shorten the
                # final drain after compute ends
                span = 2 if bt == NBT - 1 else 4
                ot = None
                o0 = 0
                for w in range(NWIN):
                    w0 = w * WIN
                    w1 = min(w0 + WIN, NCOL)
                    wl = w1 - w0
                    if w % span == 0:
                        o0 = w0
                        ot = otp.tile([BT, 4 * WIN], f16, tag="ot")

                    pm = psm.tile([BT, WIN], f32, tag="mm")
                    for (s0, s1) in _segments(w0, w1, _MM_BOUNDS):
                        f = bisect_right(_FLD_START, s0) - 1
                        g = f // 2
                        nc.tensor.matmul(
                            pm[:, s0 - w0:s1 - w0],
                            xT[:, g * BT:(g + 1) * BT],
                            wslice(f, s0 - _FLD_START[f], s1 - s0),
                            start=True, stop=True)

                    if w in PATH_A:
                        src = pm
                        eng = nc.vector
                    else:
                        src = mmp.tile([BT, WIN], f16, tag="m16")
                        nc.scalar.copy(src[:, :wl], pm[:, :wl])
                        eng = nc.gpsimd if w in GPS_W else nc.vector
                    for (s0, s1) in _segments(w0, w1, _FLD_BOUNDS):
                        i = bisect_right(_FLD_START, s0) - 1
                        xc = (i + 1) * D + (s0 - _FLD_START[i])
                        eng.tensor_mul(
                            ot[:, s0 - o0:s1 - o0],
                            src[:, s0 - w0:s1 - w0],
                            x16[:, xc:xc + (s1 - s0)])

                    if w % span == span - 1 or w == NWIN - 1:
                        ol = w1 - o0
                        nc.gpsimd.dma_start(out=out[r0:r1, o0:o0 + ol],
                                            in_=ot[:, :ol])
    nc.compile()
    return nc


def _get_nc():
    global _nc_cache
    if _nc_cache is None:
        _nc_cache = _build()
    return _nc_cache


def _prep_inputs(x, W):
    x16 = np.asarray(x, dtype=np.float16)            # (B, F, D)
    xs = np.ascontiguousarray(x16.reshape(N_CORES, BL, F * D))
    # xt[c, h*64+d, bt*2048 + g*128 + b] = x[c, bt*128+b, 2g+h, d]
    xr = x16.reshape(N_CORES, NBT, BT, NGRP, 2, D)
    xt = np.ascontiguousarray(xr.transpose(0, 4, 5, 1, 3, 2)).reshape(
        N_CORES, 128, NBT * NGRP * BT)
    # canonical W^T: wtc[d, p*64+e] = W[p, e, d]
    wtc = np.ascontiguousarray(
        np.asarray(W, dtype=np.float32).transpose(2, 0, 1).reshape(D, NCOL)
    ).astype(np.float16)
    # a-tiles (groups 0:3) zero-padded to 128 rows; b-tiles unpadded
    wta = np.zeros((128, _EA + _OA), dtype=np.float16)
    wtb = np.empty((D, (_EW - _EA) + (_OW - _OA)), dtype=np.float16)
    for f in range(F - 1):
        blk = wtc[:, _FLD_START[f]:_FLD_START[f] + _FLD_W[f]]
        if f < 2 * GSPLIT:  # a-tiles
            if f % 2 == 0:
                wta[0:D, _FLD_BASE[f]:_FLD_BASE[f] + _FLD_W[f]] = blk
            else:
                wta[D:128, _EA + _FLD_BASE[f]:
                    _EA + _FLD_BASE[f] + _FLD_W[f]] = blk
        else:
            b0 = (_FLD_BASE[f] - _EA if f % 2 == 0
                  else (_EW - _EA) + _FLD_BASE[f] - _OA)
            wtb[:, b0:b0 + _FLD_W[f]] = blk
    return xs, xt, wta, wtb


def _run(x, W, trace=False, trace_kwargs=None):
    xs, xt, wta, wtb = _prep_inputs(x, W)
    in_maps = [{"x16": xs[c], "xt": xt[c], "wta": wta, "wtb": wtb}
               for c in range(N_CORES)]
    res = run_bass_kernel_spmd(_get_nc(), in_maps, list(range(N_CORES)),
                               trace=trace, **(trace_kwargs or {}))
    outs = [res.results[c]["out"].astype(np.float32).reshape(BL, P, D)
            for c in range(N_CORES)]
    return np.concatenate(outs, axis=0), res


def kernel(x, W):
    out, _ = _run(x, W)
    return out


# revision 41
# speedup vs baseline: 1.1394x; 1.0118x over previous
"""Trainium2 Bass kernel for nn_BiLinearInteractionLayer (fp16 fast path).

Math: x:(B=4096, F=32, D=64) f32, W:(P=496, D=64, D=64) f32 (torch Linear
layout).  For each pair p=(i,j), i<j:
    out[b, p, e] = (sum_d x[b,i,d] * W[p,e,d]) * x[b,j,e]

The harness gate is rel_err < 2e-2 (max-abs / max-scale).  The original
kernel computed to 2.7e-7 with an exact hi/lo fp16 expansion and stored
fp32 output -- but it is HBM-bound (65 of 77 MB/core is the output
store).  This version computes in fp16 (~8e-4 rel err, ~25x inside the
gate) and halves the dominant traffic:

  per core: x fp16 2MB + xT fp16 2MB + W^T fp16 3.9MB + out fp16 32.5MB
  = 40.4MB vs 77MB before, at the ~358 GB/s HBM-per-core limit.

Design (data-parallel over batch, 8 cores x 512 rows):

* Host precomputes fp16 x in natural layout, fp16 x^T per batch tile in
  a per-field-pair layout (field 2g on rows 0:63, field 2g+1 on rows
  64:127), and fp16 W^T -- no on-chip transposes.
* Matmuls are k=128: the stationary is one PAIR of fields' x^T, and the
  streamed weight tile has the other field's 64 rows zeroed.  k=128
  lets the PE HAM monitor un-throttle to 2.4 GHz (k=64 under-reports
  and pins 1.2 GHz -- an all-k=64 variant measured +55us), and
  LDWEIGHTS overlaps matmuls.  Row-group packing (tile_position) was
  rejected: two concurrent row-group matmuls draining into one PSUM
  bank are a fatal HW error (verified by bisection) and the canonical
  output layout can't keep them bank-disjoint.
* Weights live in 4 SBUF tiles (even/odd fields x groups 0-3 / 4-15,
  data on partitions 0:63 / 64:127 resp.), each zero-half initialized
  by ONE big DVE memset (~3us) and filled by ONE DMA.  An earlier
  variant used 31 per-group memsets + 31 DMAs: the DVE DRAIN per memset
  plus the sync ring's ~620ns per-DMA issue cost ~35us of startup
  serialization.  Splitting even/odd at group 4 gives window 0 its
  weights after ~2 DMAs while the rest stream in behind.
* TRN2 matmul can only write fp32 PSUM.  Evacuation + elementwise
  multiply, per 2048-col window (4 PSUM banks):
    path B (most windows): ACT copies PSUM->SBUF fp16 ((172+FD)/1.2GHz),
      then DVE tensor_mul fp16*fp16->fp16 at 2x_1P.
    path A (PATH_A windows): DVE multiplies straight from fp32 PSUM at
      1x (mixed-dtype tensor_tensor with an fp16 SBUF operand works).
  The split keeps ACT and DVE balanced under the DMA floor.
* Loads go on the Sync HWDGE ring; stores (2MB spans, 1MB on the final
  batch tile to shorten the tail drain) go on the GPSIMD SWDGE queue.
  Stores must not be issued from ACT (a store waiting on DVE in the
  ACT FIFO queue blocks the next window's PSUM copy -- cost ~25us in
  an earlier variant), and bt+1's loads are issued before bt's stores
  so a ready load is never parked behind a store in FIFO order.
* Measured (8 cores): ~159us HW exec, rel err 7.7e-4.  Per-core trace:
  DMA ~124us active (~40.4MB at an effective ~300-330GB/s), ACT ~99us,
  DVE ~97us, PE ~83us (HAM oscillates warm/cold on the ~2.1us window
  cadence), plus ~7us fixed framework preamble and ~12us of startup
  chain before the first window.  Tried and rejected: GPSIMD
  tensor_mul offload (+24us -- its TT is ~3x slower than DVE 2x),
  shipping all weight zeros from DRAM (+6us: +4MB on the bottleneck),
  per-group weight DMAs (31 issues at ~620ns each gate startup).
"""
import numpy as np
from bisect import bisect_right

import concourse.bacc as bacc
import concourse.tile as tile
import concourse.mybir as mybir
from concourse.bass_utils import run_bass_kernel_spmd

B = 4096
F = 32
D = 64
P = F * (F - 1) // 2  # 496
N_CORES = 8
BL = B // N_CORES     # 512 rows per core
BT = 128              # batch tile (SBUF partitions)
NBT = BL // BT        # 4 batch tiles per core
NCOL = P * D          # 31744 output columns per row
NGRP = F // 2         # 16 field-pair groups
GSPLIT = 4            # weight tiles split at this group
WIN = 2048            # evacuation window = 4 fp32 PSUM banks
NWIN = (NCOL + WIN - 1) // WIN  # 16 (last window 1024)
PATH_A = (4, 9, 14)   # windows multiplied straight from PSUM (DVE 1x)
GPS_W = ()            # gpsimd mul offload: measured WORSE (183 vs 159us)

f32 = mybir.dt.float32
f16 = mybir.dt.float16

_nc_cache = None


def _off(i):
    """Number of pairs with left field < i."""
    return 31 * i - i * (i - 1) // 2


_FLD_START = [_off(i) * D for i in range(F)]            # canonical col starts
_FLD_W = [(31 - i) * D for i in range(F)]               # field block widths
# weight DRAM layout: even-field blocks in group order, then odd-field
# blocks in group order; _FLD_BASE = col of each field inside its e/o block
_FLD_BASE = [0] * F
for _f in range(2, F):
    _FLD_BASE[_f] = _FLD_BASE[_f - 2] + _FLD_W[_f - 2]
_EW = sum(_FLD_W[0::2])                                 # 16384 even cols
_OW = sum(_FLD_W[1::2])                                 # 15360 odd cols
_EA = _FLD_BASE[2 * GSPLIT]                             # even split col
_OA = _FLD_BASE[2 * GSPLIT + 1]                         # odd split col
_MM_BOUNDS = sorted(set(range(0, NCOL, 512)) | set(_FLD_START))
_FLD_BOUNDS = _FLD_START[1:]


def _segments(w0, w1, bounds):
    pts = [w0] + [b for b in bounds if w0 < b < w1] + [w1]
    return list(zip(pts[:-1], pts[1:]))


def _build():
    nc = bacc.Bacc("TRN2", target_bir_lowering=False, debug=False,
                   num_devices=N_CORES)
    x_in = nc.dram_tensor("x16", [BL, F * D], f16, kind="ExternalInput").ap()
    xt_in = nc.dram_tensor("xt", [128, NBT * NGRP * BT], f16,
                           kind="ExternalInput").ap()
    # cols [0:_EW) = even-field blocks, [_EW:) = odd-field blocks
    wt_in = nc.dram_tensor("wt", [D, _EW + _OW], f16,
                           kind="ExternalInput").ap()
    out = nc.dram_tensor("out", [BL, NCOL], f16, kind="ExternalOutput").ap()

    with tile.TileContext(nc) as tc:
        with (
            tc.tile_pool(name="consts", bufs=1) as consts,
            tc.tile_pool(name="xp", bufs=2) as xp,
            tc.tile_pool(name="xtp", bufs=2) as xtp,
            tc.tile_pool(name="mmp", bufs=3) as mmp,
            tc.tile_pool(name="otp", bufs=3) as otp,
            tc.tile_pool(name="psm", bufs=2, space="PSUM") as psm,
        ):
            # 4 weight tiles: (even|odd fields) x (groups 0:3 | 4:15).
            # Even-field data on partitions 0:63 (matching x^T rows), odd
            # on 64:127; the complementary half-rows are zero.  Shipping
            # the zeros from DRAM instead costs +4MB on the bottleneck
            # DMA (+13us measured) -- memsets are cheaper.
            tEa = consts.tile([128, _EA], f16, tag="wEa")
            tEb = consts.tile([128, _EW - _EA], f16, tag="wEb")
            tOa = consts.tile([128, _OA], f16, tag="wOa")
            tOb = consts.tile([128, _OW - _OA], f16, tag="wOb")
            # fp16 memset runs at 1x; bitcast to f32 halves the element
            # count.  Even-tile zeros on DVE, odd-tile zeros on the (idle)
            # GPSIMD engine so both halves clear concurrently.
            nc.vector.memset(tEa[64:128, :].bitcast(f32), 0.0)
            nc.gpsimd.memset(tOa[0:64, :].bitcast(f32), 0.0)
            nc.vector.memset(tEb[64:128, :].bitcast(f32), 0.0)
            nc.gpsimd.memset(tOb[0:64, :].bitcast(f32), 0.0)

            def wslice(f, lc, n):
                """Weight-tile slice for cols [lc, lc+n) of field f's block."""
                b = _FLD_BASE[f]
                if f % 2 == 0:
                    t, b0 = (tEa, 0) if b < _EA else (tEb, _EA)
                    return t[:, b - b0 + lc:b - b0 + lc + n]
                t, b0 = (tOa, 0) if b < _OA else (tOb, _OA)
                return t[:, b - b0 + lc:b - b0 + lc + n]

            def load_bt(bt):
                x16 = xp.tile([BT, F * D], f16, tag="x")
                nc.sync.dma_start(out=x16,
                                  in_=x_in[bt * BT:(bt + 1) * BT, :])
                c0 = bt * NGRP * BT
                xT = xtp.tile([128, NGRP * BT], f16, tag="xT")
                nc.sync.dma_start(out=xT, in_=xt_in[:, c0:c0 + NGRP * BT])
                return x16, xT

            tiles = load_bt(0)
            nc.sync.dma_start(out=tEa[0:64, :], in_=wt_in[:, 0:_EA])
            nc.sync.dma_start(out=tOa[64:128, :],
                              in_=wt_in[:, _EW:_EW + _OA])
            nc.sync.dma_start(out=tEb[0:64, :], in_=wt_in[:, _EA:_EW])
            nc.sync.dma_start(out=tOb[64:128, :],
                              in_=wt_in[:, _EW + _OA:_EW + _OW])

            for bt in range(NBT):
                r0, r1 = bt * BT, (bt + 1) * BT
                x16, xT = tiles
                if bt + 1 < NBT:
                    tiles = load_bt(bt + 1)

                # stores span 4 windows (2MB) for SWDGE efficiency; the
                # last batch tile stores every 2 windows to shorten the
                # final drain after compute ends
                span = 2 if bt == NBT - 1 else 4
                ot = None
                o0 = 0
                for w in range(NWIN):
                    w0 = w * WIN
                    w1 = min(w0 + WIN, NCOL)
                    wl = w1 - w0
                    if w % span == 0:
                        o0 = w0
                        ot = otp.tile([BT, 4 * WIN], f16, tag="ot")

                    pm = psm.tile([BT, WIN], f32, tag="mm")
                    for (s0, s1) in _segments(w0, w1, _MM_BOUNDS):
                        f = bisect_right(_FLD_START, s0) - 1
                        g = f // 2
                        nc.tensor.matmul(
                            pm[:, s0 - w0:s1 - w0],
                            xT[:, g * BT:(g + 1) * BT],
                            wslice(f, s0 - _FLD_START[f], s1 - s0),
                            start=True, stop=True)

                    if w in PATH_A:
                        src = pm
                        eng = nc.vector
                    else:
                        src = mmp.tile([BT, WIN], f16, tag="m16")
                        nc.scalar.copy(src[:, :wl], pm[:, :wl])
                        eng = nc.gpsimd if w in GPS_W else nc.vector
                    for (s0, s1) in _segments(w0, w1, _FLD_BOUNDS):
                        i = bisect_right(_FLD_START, s0) - 1
                        xc = (i + 1) * D + (s0 - _FLD_START[i])
                        eng.tensor_mul(
                            ot[:, s0 - o0:s1 - o0],
                            src[:, s0 - w0:s1 - w0],
                            x16[:, xc:xc + (s1 - s0)])

                    if w % span == span - 1 or w == NWIN - 1:
                        ol = w1 - o0
                        nc.gpsimd.dma_start(out=out[r0:r1, o0:o0 + ol],
                                            in_=ot[:, :ol])
    nc.compile()
    return nc


def _get_nc():
    global _nc_cache
    if _nc_cache is None:
        _nc_cache = _build()
    return _nc_cache


def _prep_inputs(x, W):
    x16 = np.asarray(x, dtype=np.float16)            # (B, F, D)
    xs = np.ascontiguousarray(x16.reshape(N_CORES, BL, F * D))
    # xt[c, h*64+d, bt*2048 + g*128 + b] = x[c, bt*128+b, 2g+h, d]
    xr = x16.reshape(N_CORES, NBT, BT, NGRP, 2, D)
    xt = np.ascontiguousarray(xr.transpose(0, 4, 5, 1, 3, 2)).reshape(
        N_CORES, 128, NBT * NGRP * BT)
    # canonical W^T: wtc[d, p*64+e] = W[p, e, d]
    wtc = np.ascontiguousarray(
        np.asarray(W, dtype=np.float32).transpose(2, 0, 1).reshape(D, NCOL)
    ).astype(np.float16)
    # even-field blocks then odd-field blocks
    wt2 = np.empty((D, _EW + _OW), dtype=np.float16)
    for f in range(F - 1):
        b0 = _FLD_BASE[f] + (0 if f % 2 == 0 else _EW)
        wt2[:, b0:b0 + _FLD_W[f]] = \
            wtc[:, _FLD_START[f]:_FLD_START[f] + _FLD_W[f]]
    return xs, xt, wt2


def _run(x, W, trace=False, trace_kwargs=None):
    xs, xt, wt2 = _prep_inputs(x, W)
    in_maps = [{"x16": xs[c], "xt": xt[c], "wt": wt2} for c in range(N_CORES)]
    res = run_bass_kernel_spmd(_get_nc(), in_maps, list(range(N_CORES)),
                               trace=trace, **(trace_kwargs or {}))
    outs = [res.results[c]["out"].astype(np.float32).reshape(BL, P, D)
            for c in range(N_CORES)]
    return np.concatenate(outs, axis=0), res


def kernel(x, W):
    out, _ = _run(x, W)
    return out
